# revision 24
# baseline (speedup 1.0000x reference)
"""Auditory spectrogram kernel for Trainium2 (8 NeuronCores, Bass/Tile).

Pipeline per the reference:
  y1 = order-4 IIR cochlear filterbank (129 channels, per-channel B/A) over wav [8, 64000]
  y2 = sigmoid(y1); y2 = 1st-order IIR (beta) over time
  y4 = relu(y2[c] - y2[c-1]); y5 = 1st-order IIR (alpha); downsample every 256 -> [8, 129, 250]

Strategy: all linear recurrences are restructured as blocked FIR matmuls on
TensorE (impulse responses decay geometrically; truncation tails < 1e-5).
Channel 0's output is exactly zero (diffs with itself), so the 128 real output
channels are sharded 16 per core; each core computes S1+sigmoid for its 16
channels plus a 1-channel halo.

Layout: time is blocked into 500 blocks of 128; partition dim = position in
block, free dim = (batch, block). S1 = per-channel banded-Toeplitz matmuls;
sigmoid on ScalarE; channel diff on VectorE; hair-cell LPF = 2 shared-Toeplitz
matmuls; relu on VectorE; temporal integration = weighted per-block reductions
(3 accumulating matmuls into one psum row) + a frame-rate tensor_tensor_scan.
Everything is fp16 operands with fp32 PSUM accumulation (measured end-to-end
error ~8e-4 of output absmax, reference's own fp32 noise is ~1e-4).
"""

import numpy as np

NCH, BS, T = 129, 8, 64000
L = 128                      # time block
NBLK = T // L                # 500 blocks
NFRM = 250                   # output frames (stride 256)
NCORE = 8
CPC = 16                     # output channels per core
BETA = float(np.exp(-1.0 / 8.0))
ALPHA = float(np.exp(-1.0 / 128.0))
A256 = float(ALPHA ** 256)
KMAX = 1024
TAIL_TOL = 3e-4

_cache = {}


def _impulse_responses(coch_B, coch_A):
    """h[c, k] for k < KMAX, float64, from the order-4 IIR coefficients."""
    B = coch_B.astype(np.float64)
    A = coch_A.astype(np.float64)
    h = np.zeros((NCH, KMAX))
    for t in range(KMAX):
        acc = B[:, t].copy() if t < 5 else np.zeros(NCH)
        for k in range(1, 5):
            if t - k >= 0:
                acc -= A[:, k] * h[:, t - k]
        h[:, t] = acc
    return h


def _band_matrix(hc, b):
    """T_b[p_in, p_out] = h[128*b + p_out - p_in] (0 where the tap index < 0)."""
    p = np.arange(L)
    idx = 128 * b + p[None, :] - p[:, None]
    valid = idx >= 0
    out = np.where(valid, hc[np.clip(idx, 0, KMAX - 1)], 0.0)
    return out


def _host_prep(wavData, coch_B, coch_A):
    wavData = np.asarray(wavData, dtype=np.float32)
    coch_B = np.asarray(coch_B, dtype=np.float64)
    coch_A = np.asarray(coch_A, dtype=np.float64)
    h = _impulse_responses(coch_B, coch_A)
    tails = np.cumsum(np.abs(h[:, ::-1]), axis=1)[:, ::-1]
    taps = np.array([
        int(np.argmax(tails[c] < TAIL_TOL)) if tails[c, 0] >= TAIL_TOL else 1
        for c in range(NCH)
    ])
    nb = np.clip(np.ceil(taps / 128.0).astype(int), 1, 8)
    # SPMD: every core runs the same program, so band counts must be uniform
    # per local channel position (max across cores).
    nb_u = [max(int(nb[CPC * k + i]) for k in range(NCORE)) for i in range(CPC + 1)]
    nbtot = sum(nb_u)
    woff = np.cumsum([0] + nb_u)

    # x: [128 pos, (bs, block)] fp16, same for all cores
    x16 = np.ascontiguousarray(
        np.asarray(wavData, dtype=np.float32).reshape(BS, NBLK, L)
        .transpose(2, 0, 1).reshape(L, BS * NBLK)
    ).astype(np.float16)

    w1s = []
    for k in range(NCORE):
        W1 = np.zeros((L, nbtot * L), np.float16)
        for i in range(CPC + 1):
            c = CPC * k + i
            for b in range(nb_u[i]):
                W1[:, (woff[i] + b) * L:(woff[i] + b + 1) * L] = \
                    _band_matrix(h[c], b).astype(np.float16)
        w1s.append(W1)

    p = np.arange(L)
    T0 = np.where(p[None, :] >= p[:, None], BETA ** (p[None, :] - p[:, None]), 0.0)
    T1 = np.where(p[:, None] > p[None, :], BETA ** (128 + p[None, :] - p[:, None]), 0.0)
    WB = np.concatenate([T0, T1], axis=1).astype(np.float16)

    W3 = np.zeros((L, 3), np.float16)
    W3[0, 0] = 1.0                                  # e0: the frame sample itself
    W3[1:, 1] = (ALPHA ** (256 - p[1:])).astype(np.float16)   # prev-prev block
    W3[:, 2] = (ALPHA ** (128 - p)).astype(np.float16)        # prev block
    EYE = np.eye(L, dtype=np.float16).reshape(1, L * L)
    return x16, w1s, WB, W3, EYE, tuple(nb_u)


def _build(nb_u, nrep=1, dyn_rep=1, accum_probe=False, stage='full'):
    import contextlib
    import concourse.bacc as bacc
    import concourse.tile as tile
    from concourse import mybir

    nbtot = sum(nb_u)
    woff = np.cumsum([0] + list(nb_u))
    f16, f32 = mybir.dt.float16, mybir.dt.float32

    nc = bacc.Bacc("TRN2", target_bir_lowering=False, debug=False,
                   num_devices=NCORE)
    x_d = nc.dram_tensor("x", [L, BS * NBLK], f16, kind="ExternalInput")
    w1_d = nc.dram_tensor("w1", [L, nbtot * L], f16, kind="ExternalInput")
    wb_d = nc.dram_tensor("wb", [L, 256], f16, kind="ExternalInput")
    w3_d = nc.dram_tensor("w3", [L, 3], f16, kind="ExternalInput")
    eye_d = nc.dram_tensor("eye", [1, L * L], f16, kind="ExternalInput")
    out_d = nc.dram_tensor("out", [L, NFRM], f32, kind="ExternalOutput")

    with tile.TileContext(nc) as tc:
        with tc.tile_pool(name="const", bufs=1) as cp, \
             tc.tile_pool(name="work", bufs=4) as wp, \
             tc.tile_pool(name="ps1", bufs=1, space="PSUM") as ps1p, \
             tc.tile_pool(name="ps2", bufs=2, space="PSUM") as ps2p, \
             tc.tile_pool(name="ps3", bufs=1, space="PSUM") as ps3p, \
             tc.tile_pool(name="psg", bufs=1, space="PSUM") as psgp:
            x_sb = cp.tile([L, BS * NBLK], f16, name="x_sb")
            w1_sb = cp.tile([L, nbtot * L], f16, name="w1_sb")
            wb_sb = cp.tile([L, 256], f16, name="wb_sb")
            w3_sb = cp.tile([L, 3], f16, name="w3_sb")
            eye_sb = cp.tile([1, L * L], f16, name="eye_sb")
            acst = cp.tile([L, NFRM], f32, name="acst")
            s_a = cp.tile([L, BS * NBLK], f16, name="s_a")
            s_b = cp.tile([L, BS * NBLK], f16, name="s_b")
            f_sb = cp.tile([L, NFRM], f32, name="f_sb")
            f_acc = cp.tile([L, NFRM], f32, name="f_acc")

            nc.sync.dma_start(x_sb[:], x_d.ap())
            nc.sync.dma_start(w1_sb[:], w1_d.ap())
            nc.sync.dma_start(wb_sb[:], wb_d.ap())
            nc.sync.dma_start(w3_sb[:], w3_d.ap())
            nc.sync.dma_start(eye_sb[:], eye_d.ap())
            nc.vector.memset(acst[:], A256)
            nc.vector.memset(f_acc[:], 0.0)

            loop_ctx = (tc.For_i(0, dyn_rep, 1) if dyn_rep > 1
                        else contextlib.nullcontext())
            with loop_ctx:
              for rep in range(nrep):
                psum_g = psgp.tile([L, NFRM], f32, name=f"psum_g_{rep}",
                                   tag="gather")
                for ci in range(CPC + 1):
                    s_cur = (s_a, s_b)[ci % 2]
                    s_prev = (s_a, s_b)[(ci + 1) % 2]
                    nb = nb_u[ci]
                    # S1: cochlear FIR, 4 batch lanes at a time (4 psum banks)
                    for half in range(2):
                        pss = [
                            ps1p.tile([L, NBLK], f32,
                                      name=f"ps1_{rep}_{ci}_{half}_{i}",
                                      tag=f"s1_{i}")
                            for i in range(4)
                        ]
                        for b in range(nb):
                            wap = w1_sb[:, (woff[ci] + b) * L:
                                        (woff[ci] + b + 1) * L]
                            for i in range(4):
                                bs = half * 4 + i
                                nc.tensor.matmul(
                                    pss[i][:, b:NBLK], wap,
                                    x_sb[:, bs * NBLK: bs * NBLK + NBLK - b],
                                    start=(b == 0), stop=(b == nb - 1))
                        for i in range(4):
                            bs = half * 4 + i
                            nc.scalar.activation(
                                s_cur[:, bs * NBLK:(bs + 1) * NBLK], pss[i][:],
                                mybir.ActivationFunctionType.Sigmoid)
                    if ci == 0 or stage == 's1':
                        continue
                    # S2 + S3, two (channel, batch) pairs at a time so the
                    # T0/T1 weight loads amortize over 2 matmuls each
                    for bs2 in range(0, BS, 2):
                        rr = [(ci - 1) * BS + bs2 + j for j in range(2)]
                        dd, pp2 = [], []
                        for j in range(2):
                            bs = bs2 + j
                            sl = slice(bs * NBLK, (bs + 1) * NBLK)
                            d = wp.tile([L, NBLK], f16,
                                        name=f"d_{rep}_{rr[j]}", tag="d")
                            nc.vector.tensor_sub(d[:], s_cur[:, sl],
                                                 s_prev[:, sl])
                            dd.append(d)
                            pp2.append(ps2p.tile([L, NBLK], f32,
                                                 name=f"p2_{rep}_{rr[j]}",
                                                 tag="s2"))
                        for j in range(2):
                            nc.tensor.matmul(pp2[j][:, 0:NBLK], wb_sb[:, 0:L],
                                             dd[j][:, 0:NBLK],
                                             start=True, stop=False)
                        for j in range(2):
                            nc.tensor.matmul(pp2[j][:, 1:NBLK],
                                             wb_sb[:, L:2 * L],
                                             dd[j][:, 0:NBLK - 1],
                                             start=False, stop=True)
                        for j in range(2):
                            r = rr[j]
                            y4 = wp.tile([L, NBLK], f16, name=f"y4_{rep}_{r}",
                                         tag="y4")
                            nc.vector.tensor_scalar_max(y4[:], pp2[j][:], 0.0)
                            if stage == 's2':
                                continue
                            pg = ps3p.tile([1, NFRM], f32, name=f"pg_{rep}_{r}",
                                           tag="g")
                            nc.tensor.matmul(pg[0:1, 0:NFRM], w3_sb[:, 0:1],
                                             y4[:, 0:2 * NFRM:2],
                                             start=True, stop=False)
                            nc.tensor.matmul(pg[0:1, 1:NFRM], w3_sb[:, 1:2],
                                             y4[:, 0:2 * NFRM - 2:2],
                                             start=False, stop=False)
                            nc.tensor.matmul(pg[0:1, 1:NFRM], w3_sb[:, 2:3],
                                             y4[:, 1:2 * NFRM - 1:2],
                                             start=False, stop=True)
                            # G row r -> fp16, then scatter into psum_g row r
                            # via a K=1 indicator matmul (engines can't write
                            # rows at arbitrary partitions; the PE can)
                            if stage == 's3':
                                continue
                            g_p = wp.tile([1, NFRM], f16, name=f"g_{rep}_{r}",
                                          tag="gp")
                            nc.scalar.copy(g_p[0:1, :], pg[0:1, :])
                            nc.tensor.matmul(
                                psum_g[:, 0:NFRM],
                                eye_sb[0:1, r * L:(r + 1) * L],
                                g_p[0:1, 0:NFRM],
                                start=(r == 0), stop=(r == L - 1))
                # temporal integration across frames for all 128 rows at once
                if stage != 'full':
                    nc.sync.dma_start(out_d.ap(), acst[:])
                    continue
                nc.vector.tensor_tensor_scan(
                    f_sb[:], acst[:], psum_g[:],
                    0.0, mybir.AluOpType.mult, mybir.AluOpType.add)
                if accum_probe:
                    nc.vector.tensor_add(f_acc[:], f_acc[:], f_sb[:])
                    nc.sync.dma_start(out_d.ap(), f_acc[:])
                else:
                    nc.sync.dma_start(out_d.ap(), f_sb[:])
    _dedupe_ldweights(nc)
    nc.compile()
    return nc


def _dedupe_ldweights(nc):
    """Drop PE weight loads whose stationary operand matches the previous
    load in the scheduled PE stream (the splitter emits one per matmul)."""
    from concourse import mybir
    dropped = 0
    for bb in nc.m.functions[0].blocks:
        last_key = None
        keep = []
        for inst in bb.instructions:
            if isinstance(inst, mybir.InstLdweights):
                si = inst.sync_info
                key = str(inst.ins[0])
                if (key == last_key and not (si and (si.on_wait or si.on_update))):
                    dropped += 1
                    continue
                last_key = key
            elif isinstance(inst, (mybir.InstUnconditionalBranch,
                                   mybir.InstCompareAndBranch)):
                last_key = None
            keep.append(inst)
        if len(keep) != len(bb.instructions):
            bb.instructions = keep
    return dropped


def _make_runner(nc):
    """Persistent jitted 8-core runner (mirrors bass2jax.run_bass_via_pjrt)."""
    import jax
    import jax.numpy as jnp  # noqa: F401
    from jax.sharding import Mesh, PartitionSpec
    from jax.experimental.shard_map import shard_map
    from concourse import bass2jax, mybir

    bass2jax.install_neuronx_cc_hook()

    partition_name = (
        nc.partition_id_tensor.name if nc.partition_id_tensor else None
    )
    in_names, out_names, out_avals, zero_shapes = [], [], [], []
    for alloc in nc.m.functions[0].allocations:
        if not isinstance(alloc, mybir.MemoryLocationSet):
            continue
        name = alloc.memorylocations[0].name
        if alloc.kind == "ExternalInput":
            if name != partition_name:
                in_names.append(name)
        elif alloc.kind == "ExternalOutput":
            out_names.append(name)
            shape = tuple(alloc.tensor_shape)
            dtype = mybir.dt.np(alloc.dtype)
            out_avals.append(jax.core.ShapedArray(shape, dtype))
            zero_shapes.append((shape, dtype))
    n_params = len(in_names)
    all_in_names = list(in_names) + list(out_names)
    if partition_name is not None:
        all_in_names.append(partition_name)

    def _body(*args):
        operands = list(args)
        if partition_name is not None:
            operands.append(bass2jax.partition_id_tensor())
        outs = bass2jax._bass_exec_p.bind(
            *operands,
            out_avals=tuple(out_avals),
            in_names=tuple(all_in_names),
            out_names=tuple(out_names),
            lowering_input_output_aliases=(),
            sim_require_finite=True,
            sim_require_nnan=True,
            nc=nc,
        )
        return tuple(outs)

    devices = jax.devices()[:NCORE]
    mesh = Mesh(np.asarray(devices), ("core",))
    n_outs = len(out_names)
    sharded = jax.jit(
        shard_map(_body, mesh=mesh,
                  in_specs=(PartitionSpec("core"),) * (n_params + n_outs),
                  out_specs=(PartitionSpec("core"),) * n_outs,
                  check_rep=False),
        donate_argnums=tuple(range(n_params, n_params + n_outs)),
        keep_unused=True,
    )

    def run(in_maps):
        concat_in = [
            np.concatenate([np.asarray(m[name]) for m in in_maps], axis=0)
            for name in in_names
        ]
        concat_zeros = [
            np.zeros((NCORE * s[0], *s[1:]), d) for (s, d) in zero_shapes
        ]
        out_arrs = sharded(*concat_in, *concat_zeros)
        return [
            {name: np.asarray(out_arrs[i]).reshape(NCORE, *out_avals[i].shape)[c]
             for i, name in enumerate(out_names)}
            for c in range(NCORE)
        ]

    return run


def _get_runner(wavData, coch_B, coch_A):
    x16, w1s, WB, W3, EYE, nb_u = _host_prep(wavData, coch_B, coch_A)
    if nb_u not in _cache:
        nc = _build(nb_u)
        _cache[nb_u] = _make_runner(nc)
    run = _cache[nb_u]
    in_maps = [dict(x=x16, w1=w1s[k], wb=WB, w3=W3, eye=EYE)
               for k in range(NCORE)]
    return run, in_maps


def kernel(wavData, coch_B, coch_A):
    run, in_maps = _get_runner(wavData, coch_B, coch_A)
    results = run(in_maps)
    out = np.zeros((BS, NCH, NFRM), np.float32)
    for k in range(NCORE):
        F = results[k]["out"]                      # [128, 250]
        out[:, CPC * k + 1: CPC * (k + 1) + 1, :] = \
            F.reshape(CPC, BS, NFRM).transpose(1, 0, 2)
    return out
